# revision 1
# baseline (speedup 1.0000x reference)
"""LoRA QKV parallel linear with per-token slot routing, on 8 TRN2 NeuronCores.

Strategy: data-parallel over the token dim T (8192 -> 1024 tokens/core),
weights replicated. Per core, everything is one fused pass:

  y[t, o] = sum_k x[t,k] W[o,k]  +  sum_{l,r} mask[t,l] * h[t, g(o), l, r] * B_g[l, o, r]

where h = x @ A^T for all 8 slots (dense; 384 extra contraction cols is ~19%
more FLOPs than routing but fully regular), and the routing is applied as an
on-device one-hot mask (is_equal against a constant slot-id tile). The LoRA
scaling is folded into the B matrices host-side.

All matmul operands are pre-transposed host-side so the contraction dim K is
on SBUF partitions, and use float32r (full-rate fp32 via transpose mode).
"""

import numpy as np

import concourse.bass as bass
import concourse.bacc as bacc
import concourse.mybir as mybir
import concourse.tile as tile
from concourse.masks import make_identity

HIDDEN = 2048
Q_SIZE = 2048
KV_SIZE = 512
OUT = Q_SIZE + 2 * KV_SIZE  # 3072
MAX_LORAS = 8
RANK = 16
T = 8192
N_CORES = 8
T_CORE = T // N_CORES  # 1024

P = 128
NT = T_CORE // P          # 8 token tiles per core
KC = HIDDEN // P          # 16 k-chunks
OJ = OUT // 512           # 6 output chunks of 512
GR = MAX_LORAS * RANK     # 128 = all slots*ranks for one target group
F32 = mybir.dt.float32
F32R = mybir.dt.float32r

_NC_CACHE = {}


def build_nc(mm_dtype=F32R):
    """Build the SPMD Bass program (same program on every core)."""
    nc = bacc.Bacc("TRN2", target_bir_lowering=False, debug=False, num_devices=N_CORES)

    xT = nc.dram_tensor("xT", [HIDDEN, T_CORE], F32R, kind="ExternalInput").ap()
    wT = nc.dram_tensor("wT", [HIDDEN, OUT], F32R, kind="ExternalInput").ap()
    aT = nc.dram_tensor("aT", [HIDDEN, 3 * GR], F32R, kind="ExternalInput").ap()
    bq = nc.dram_tensor("bq", [GR, Q_SIZE], F32R, kind="ExternalInput").ap()
    bk = nc.dram_tensor("bk", [GR, KV_SIZE], F32R, kind="ExternalInput").ap()
    bv = nc.dram_tensor("bv", [GR, KV_SIZE], F32R, kind="ExternalInput").ap()
    slotf = nc.dram_tensor("slotf", [T_CORE, 1], F32, kind="ExternalInput").ap()
    y = nc.dram_tensor("y", [T_CORE, OUT], F32, kind="ExternalOutput").ap()

    with tile.TileContext(nc) as tc:
        with (
            tc.tile_pool(name="const", bufs=1) as const_pool,
            tc.tile_pool(name="xsb", bufs=1) as xpool,
            tc.tile_pool(name="asb", bufs=1) as apool,
            tc.tile_pool(name="bsb", bufs=1) as bpool,
            tc.tile_pool(name="hT", bufs=1) as htpool,
            tc.tile_pool(name="m", bufs=2) as mpool,
            tc.tile_pool(name="hm", bufs=2) as hmpool,
            tc.tile_pool(name="w", bufs=3) as wpool,
            tc.tile_pool(name="o", bufs=4) as opool,
            tc.tile_pool(name="hps", bufs=2, space="PSUM") as hpsum,
            tc.tile_pool(name="tps", bufs=2, space="PSUM") as tpsum,
            tc.tile_pool(name="ops", bufs=4, space="PSUM") as opsum,
        ):
            # --- constants ---
            ident = const_pool.tile([P, P], F32)
            make_identity(nc, ident)
            c128 = const_pool.tile([P, P], F32)  # c128[t, l*16+r] = l
            for l in range(MAX_LORAS):
                nc.vector.memset(c128[:, l * RANK:(l + 1) * RANK], float(l))
            slot_sb = const_pool.tile([P, NT], F32)  # col i = token tile i
            nc.sync.dma_start(
                slot_sb[:], slotf.rearrange("(n p) o -> p (n o)", p=P))

            # --- resident inputs ---
            xsb = xpool.tile([P, KC * T_CORE], F32R)  # free idx = k*T_CORE + t
            for k in range(KC):
                nc.sync.dma_start(
                    xsb[:, k * T_CORE:(k + 1) * T_CORE],
                    xT[k * P:(k + 1) * P, :])
            asb = apool.tile([P, KC * 3 * GR], F32R)  # free idx = k*384 + (g*128+l*16+r)
            for k in range(KC):
                nc.sync.dma_start(
                    asb[:, k * 3 * GR:(k + 1) * 3 * GR],
                    aT[k * P:(k + 1) * P, :])
            bqsb = bpool.tile([P, Q_SIZE], F32R)
            bksb = bpool.tile([P, KV_SIZE], F32R)
            bvsb = bpool.tile([P, KV_SIZE], F32R)
            nc.sync.dma_start(bqsb[:], bq[:, :])
            nc.sync.dma_start(bksb[:], bk[:, :])
            nc.sync.dma_start(bvsb[:], bv[:, :])

            # --- phase 1: h = x@A^T per token tile, mask, transpose ---
            # hT_sb[:, i*384 + g*128 : ...] = (mask * h_g)^T for token tile i
            hT_sb = htpool.tile([P, NT * 3 * GR], F32R)
            for i in range(NT):
                hps = hpsum.tile([P, 3 * GR], F32)
                for k in range(KC):
                    nc.tensor.matmul(
                        hps[:],
                        lhsT=xsb[:, k * T_CORE + i * P: k * T_CORE + (i + 1) * P],
                        rhs=asb[:, k * 3 * GR:(k + 1) * 3 * GR],
                        start=(k == 0), stop=(k == KC - 1))
                m128 = mpool.tile([P, P], F32)  # one-hot routing mask
                nc.vector.tensor_scalar(
                    m128[:], c128[:], slot_sb[:, i:i + 1], None,
                    op0=mybir.AluOpType.is_equal)
                hm = hmpool.tile([P, 3 * GR], F32)
                for g in range(3):
                    nc.vector.tensor_tensor(
                        hm[:, g * GR:(g + 1) * GR],
                        hps[:, g * GR:(g + 1) * GR], m128[:],
                        op=mybir.AluOpType.mult)
                for g in range(3):
                    tp = tpsum.tile([P, P], F32)
                    nc.tensor.transpose(tp[:], hm[:, g * GR:(g + 1) * GR], ident[:])
                    nc.vector.tensor_copy(
                        hT_sb[:, (i * 3 + g) * GR:(i * 3 + g + 1) * GR], tp[:])

            # --- phase 2: y = x@W^T + hm@B, streamed over 512-wide o-chunks ---
            for j in range(OJ):
                if j < Q_SIZE // 512:
                    g, bsl = 0, bqsb[:, j * 512:(j + 1) * 512]
                elif j == Q_SIZE // 512:
                    g, bsl = 1, bksb[:]
                else:
                    g, bsl = 2, bvsb[:]
                wh = []
                for half in range(2):
                    wsb = wpool.tile([P, 8 * 512], F32R, tag="w")
                    for kk in range(8):
                        k = half * 8 + kk
                        nc.sync.dma_start(
                            wsb[:, kk * 512:(kk + 1) * 512],
                            wT[k * P:(k + 1) * P, j * 512:(j + 1) * 512])
                    wh.append(wsb)
                for i in range(NT):
                    ops = opsum.tile([P, 512], F32)
                    for k in range(KC):
                        nc.tensor.matmul(
                            ops[:],
                            lhsT=xsb[:, k * T_CORE + i * P: k * T_CORE + (i + 1) * P],
                            rhs=wh[k // 8][:, (k % 8) * 512:(k % 8 + 1) * 512],
                            start=(k == 0), stop=False)
                    nc.tensor.matmul(
                        ops[:],
                        lhsT=hT_sb[:, (i * 3 + g) * GR:(i * 3 + g + 1) * GR],
                        rhs=bsl,
                        start=False, stop=True)
                    osb = opool.tile([P, 512], F32)
                    nc.scalar.copy(osb[:], ops[:])
                    nc.sync.dma_start(
                        y[i * P:(i + 1) * P, j * 512:(j + 1) * 512], osb[:])
    nc.compile()
    return nc


def prep_in_maps(x, weight, lora_A, lora_B_q, lora_B_k, lora_B_v,
                 lora_scaling, token_to_slot):
    x = np.asarray(x, dtype=np.float32)
    weight = np.asarray(weight, dtype=np.float32)
    lora_A = np.asarray(lora_A, dtype=np.float32)
    lora_B_q = np.asarray(lora_B_q, dtype=np.float32)
    lora_B_k = np.asarray(lora_B_k, dtype=np.float32)
    lora_B_v = np.asarray(lora_B_v, dtype=np.float32)
    lora_scaling = np.asarray(lora_scaling, dtype=np.float32)
    slot = np.asarray(token_to_slot)

    xT = np.ascontiguousarray(x.T)                      # (2048, 8192)
    wT = np.ascontiguousarray(weight.T)                 # (2048, 3072)
    # aT col = g*128 + l*16 + r
    aT = np.ascontiguousarray(
        lora_A.transpose(1, 0, 2, 3).reshape(3 * GR, HIDDEN).T)
    # b row = l*16 + r, with scaling folded in
    bq = np.ascontiguousarray(
        (lora_scaling[:, None, None] * lora_B_q).transpose(0, 2, 1).reshape(GR, Q_SIZE))
    bk = np.ascontiguousarray(
        (lora_scaling[:, None, None] * lora_B_k).transpose(0, 2, 1).reshape(GR, KV_SIZE))
    bv = np.ascontiguousarray(
        (lora_scaling[:, None, None] * lora_B_v).transpose(0, 2, 1).reshape(GR, KV_SIZE))
    slotf = slot.astype(np.float32).reshape(T, 1)

    in_maps = []
    for c in range(N_CORES):
        in_maps.append({
            "xT": np.ascontiguousarray(xT[:, c * T_CORE:(c + 1) * T_CORE]),
            "wT": wT,
            "aT": aT,
            "bq": bq,
            "bk": bk,
            "bv": bv,
            "slotf": np.ascontiguousarray(slotf[c * T_CORE:(c + 1) * T_CORE]),
        })
    return in_maps


def kernel(**inputs):
    from concourse.bass_utils import run_bass_kernel_spmd
    if "nc" not in _NC_CACHE:
        _NC_CACHE["nc"] = build_nc()
    nc = _NC_CACHE["nc"]
    in_maps = prep_in_maps(**inputs)
    res = run_bass_kernel_spmd(nc, in_maps, core_ids=list(range(N_CORES)))
    return np.concatenate([r["y"] for r in res.results], axis=0)



# revision 9
# speedup vs baseline: 1.1493x; 1.1493x over previous
"""LoRA QKV parallel linear with per-token slot routing, on 8 TRN2 NeuronCores.

Data-parallel over tokens (8192 -> 1024 per core), weights replicated.
All matmul operands are bf16 (tolerance 2e-2 >> bf16 matmul error ~5e-3),
which halves HBM traffic and SBUF footprint vs f32 and enables the fast
weight-load path. PSUM accumulation stays fp32.

Per core:
  phase 1: hT[(g,l,r), t] = A^T-chunk-stationary matmuls over k, directly in
           the [rank, token] orientation (no PE transposes), then one
           elementwise multiply with a host-precomputed routing mask
           m[(l,r), t] = scale[l] * (slot[t] == l)  (scale folded into B
           host-side, so m is one-hot {0,1}).
  phase 2: for each 128-token tile i: out[t, o] accumulates
           sum_k x[k,i]^T @ W[k, o] over 16 k-chunks into 6 PSUM banks
           (one per 512-wide o-chunk), closed by the LoRA delta matmul
           hT-slice-stationary @ B. PSUM -> SBUF copies alternate between
           the Scalar and Vector engines; output is written bf16 and
           upcast on host.
"""

import numpy as np
import ml_dtypes

import concourse.bass as bass
import concourse.bacc as bacc
import concourse.mybir as mybir
import concourse.tile as tile

HIDDEN = 2048
Q_SIZE = 2048
KV_SIZE = 512
OUT = Q_SIZE + 2 * KV_SIZE  # 3072
MAX_LORAS = 8
RANK = 16
T = 8192
N_CORES = 8
T_CORE = T // N_CORES  # 1024

P = 128
KC = HIDDEN // P          # 16 k-chunks
OJ = OUT // 512           # 6 output chunks of 512
NT = T_CORE // P          # 8 token tiles per core
GR = MAX_LORAS * RANK     # 128 = all (slot, rank) pairs for one target group
F32 = mybir.dt.float32
BF16 = mybir.dt.bfloat16
NPBF16 = ml_dtypes.bfloat16

_NC_CACHE = {}


def build_nc():
    """Build the SPMD Bass program (same program on every core)."""
    nc = bacc.Bacc("TRN2", target_bir_lowering=False, debug=False, num_devices=N_CORES)

    xT = nc.dram_tensor("xT", [HIDDEN, T_CORE], BF16, kind="ExternalInput").ap()
    wT = nc.dram_tensor("wT", [HIDDEN, OUT], BF16, kind="ExternalInput").ap()
    aT = nc.dram_tensor("aT", [HIDDEN, 3 * GR], BF16, kind="ExternalInput").ap()
    bqkv = nc.dram_tensor("bqkv", [GR, OUT], BF16, kind="ExternalInput").ap()
    mask = nc.dram_tensor("mask", [GR, T_CORE], BF16, kind="ExternalInput").ap()
    yb = nc.dram_tensor("yb", [T_CORE, OUT], BF16, kind="ExternalOutput").ap()

    with tile.TileContext(nc) as tc:
        with (
            tc.tile_pool(name="xsb", bufs=1) as xpool,
            tc.tile_pool(name="wsb", bufs=1) as wpool,
            tc.tile_pool(name="asb", bufs=1) as apool,
            tc.tile_pool(name="bsb", bufs=1) as bpool,
            tc.tile_pool(name="msb", bufs=1) as mpool,
            tc.tile_pool(name="ht", bufs=1) as htpool,
            tc.tile_pool(name="o", bufs=12) as opool,
            tc.tile_pool(name="ps", bufs=1, space="PSUM") as pspool,
        ):
            # --- resident inputs (DMA'd in chunks; tile deps order the MMs) ---
            xsb = xpool.tile([P, KC * T_CORE], BF16)    # free idx = k*1024 + t
            asb = apool.tile([P, KC * 3 * GR], BF16)    # free idx = k*384 + g*128 + lr
            for k in range(KC):
                nc.sync.dma_start(
                    xsb[:, k * T_CORE:(k + 1) * T_CORE], xT[k * P:(k + 1) * P, :])
                nc.sync.dma_start(
                    asb[:, k * 3 * GR:(k + 1) * 3 * GR], aT[k * P:(k + 1) * P, :])
            msb = mpool.tile([P, T_CORE], BF16)         # routing mask [lr, t]
            nc.sync.dma_start(msb[:], mask[:, :])
            bsb = bpool.tile([P, OUT], BF16)            # [lr, o] = bq|bk|bv
            nc.sync.dma_start(bsb[:], bqkv[:, :])
            wsb = wpool.tile([P, KC * OUT], BF16)       # free idx = k*3072 + o
            for k in range(KC):
                nc.sync.dma_start(
                    wsb[:, k * OUT:(k + 1) * OUT], wT[k * P:(k + 1) * P, :])

            # --- phase 1: hT[g][lr, t], A-chunk stationary, x moving ---
            hps = [pspool.tile([P, 512], F32, name=f"hps{n}", tag=f"ps{n}") for n in range(6)]
            for k in range(KC):
                for g in range(3):
                    for th in range(2):
                        nc.tensor.matmul(
                            hps[g * 2 + th][:],
                            lhsT=asb[:, k * 3 * GR + g * GR:
                                     k * 3 * GR + (g + 1) * GR],
                            rhs=xsb[:, k * T_CORE + th * 512:
                                    k * T_CORE + (th + 1) * 512],
                            start=(k == 0), stop=(k == KC - 1))
            # mask (one-hot) and downcast to bf16 for the delta matmuls
            hT = htpool.tile([P, 3 * T_CORE], BF16)     # free idx = g*1024 + t
            for g in range(3):
                for th in range(2):
                    nc.vector.tensor_tensor(
                        hT[:, g * T_CORE + th * 512:g * T_CORE + (th + 1) * 512],
                        hps[g * 2 + th][:], msb[:, th * 512:(th + 1) * 512],
                        op=mybir.AluOpType.mult)

            # --- phase 2: y tile [t128, o512] per (i, j) ---
            # delta source group per o-chunk j: q,q,q,q,k,v
            jg = [0, 0, 0, 0, 1, 2]
            for i in range(NT):
                ops = [pspool.tile([P, 512], F32, name=f"ops{n}", tag=f"ps{n}") for n in range(OJ)]
                for k in range(KC):
                    for j in range(OJ):
                        nc.tensor.matmul(
                            ops[j][:],
                            lhsT=xsb[:, k * T_CORE + i * P:
                                     k * T_CORE + (i + 1) * P],
                            rhs=wsb[:, k * OUT + j * 512:k * OUT + (j + 1) * 512],
                            start=(k == 0), stop=False)
                for j in range(OJ):
                    nc.tensor.matmul(
                        ops[j][:],
                        lhsT=hT[:, jg[j] * T_CORE + i * P:
                                jg[j] * T_CORE + (i + 1) * P],
                        rhs=bsb[:, j * 512:(j + 1) * 512],
                        start=False, stop=True)
                    osb = opool.tile([P, 512], BF16, name=f"osb{i}_{j}", tag="o")
                    if j % 2 == 0:
                        nc.scalar.copy(osb[:], ops[j][:])
                    else:
                        nc.vector.tensor_copy(osb[:], ops[j][:])
                    nc.sync.dma_start(
                        yb[i * P:(i + 1) * P, j * 512:(j + 1) * 512], osb[:])
    nc.compile()
    return nc


def prep_in_maps(x, weight, lora_A, lora_B_q, lora_B_k, lora_B_v,
                 lora_scaling, token_to_slot):
    x = np.asarray(x, dtype=np.float32)
    weight = np.asarray(weight, dtype=np.float32)
    lora_A = np.asarray(lora_A, dtype=np.float32)
    lora_B_q = np.asarray(lora_B_q, dtype=np.float32)
    lora_B_k = np.asarray(lora_B_k, dtype=np.float32)
    lora_B_v = np.asarray(lora_B_v, dtype=np.float32)
    lora_scaling = np.asarray(lora_scaling, dtype=np.float32)
    slot = np.asarray(token_to_slot).astype(np.int64)

    xT = np.ascontiguousarray(x.astype(NPBF16).T)       # (2048, 8192) bf16
    wT = np.ascontiguousarray(weight.astype(NPBF16).T)  # (2048, 3072) bf16
    # aT col = g*128 + l*16 + r
    aT = np.ascontiguousarray(
        lora_A.transpose(1, 0, 2, 3).reshape(3 * GR, HIDDEN).T.astype(NPBF16))
    # b row = l*16 + r, scaling folded in; columns = q | k | v
    bq = (lora_scaling[:, None, None] * lora_B_q).transpose(0, 2, 1).reshape(GR, Q_SIZE)
    bk = (lora_scaling[:, None, None] * lora_B_k).transpose(0, 2, 1).reshape(GR, KV_SIZE)
    bv = (lora_scaling[:, None, None] * lora_B_v).transpose(0, 2, 1).reshape(GR, KV_SIZE)
    bqkv = np.ascontiguousarray(
        np.concatenate([bq, bk, bv], axis=1).astype(NPBF16))  # (128, 3072)
    # routing mask [l*16+r, t]: 1 where slot[t] == l (scale already in B)
    onehot = (slot[None, :] == np.arange(MAX_LORAS)[:, None])          # (8, T)
    mask = np.repeat(onehot, RANK, axis=0).astype(NPBF16)              # (128, T)

    in_maps = []
    for c in range(N_CORES):
        sl = slice(c * T_CORE, (c + 1) * T_CORE)
        in_maps.append({
            "xT": np.ascontiguousarray(xT[:, sl]),
            "wT": wT,
            "aT": aT,
            "bqkv": bqkv,
            "mask": np.ascontiguousarray(mask[:, sl]),
        })
    return in_maps


def kernel(**inputs):
    from concourse.bass_utils import run_bass_kernel_spmd
    if "nc" not in _NC_CACHE:
        _NC_CACHE["nc"] = build_nc()
    nc = _NC_CACHE["nc"]
    in_maps = prep_in_maps(**inputs)
    res = run_bass_kernel_spmd(nc, in_maps, core_ids=list(range(N_CORES)))
    return np.concatenate(
        [r["yb"].astype(np.float32) for r in res.results], axis=0)


# revision 11
# speedup vs baseline: 1.2362x; 1.0756x over previous
"""LoRA QKV parallel linear with per-token slot routing, on 8 TRN2 NeuronCores.

Data-parallel over tokens (8192 -> 1024 per core), weights replicated.
All matmul operands are bf16 (tolerance 2e-2 >> bf16 matmul error ~5e-3),
which halves HBM traffic and SBUF footprint vs f32 and enables the fast
weight-load path. PSUM accumulation stays fp32.

Per core:
  phase 1: hT[(g,l,r), t] = A^T-chunk-stationary matmuls over k, directly in
           the [rank, token] orientation (no PE transposes), then one
           elementwise multiply with a host-precomputed routing mask
           m[(l,r), t] = scale[l] * (slot[t] == l)  (scale folded into B
           host-side, so m is one-hot {0,1}).
  phase 2: for each 128-token tile i: out[t, o] accumulates
           sum_k x[k,i]^T @ W[k, o] over 16 k-chunks into 6 PSUM banks
           (one per 512-wide o-chunk), closed by the LoRA delta matmul
           hT-slice-stationary @ B. PSUM -> SBUF copies alternate between
           the Scalar and Vector engines; output is written bf16 and
           upcast on host.
"""

import numpy as np
import ml_dtypes

import concourse.bass as bass
import concourse.bacc as bacc
import concourse.mybir as mybir
import concourse.tile as tile

HIDDEN = 2048
Q_SIZE = 2048
KV_SIZE = 512
OUT = Q_SIZE + 2 * KV_SIZE  # 3072
MAX_LORAS = 8
RANK = 16
T = 8192
N_CORES = 8
T_CORE = T // N_CORES  # 1024

P = 128
KC = HIDDEN // P          # 16 k-chunks
OJ = OUT // 512           # 6 output chunks of 512
NT = T_CORE // P          # 8 token tiles per core
GR = MAX_LORAS * RANK     # 128 = all (slot, rank) pairs for one target group
F32 = mybir.dt.float32
BF16 = mybir.dt.bfloat16
NPBF16 = ml_dtypes.bfloat16

_NC_CACHE = {}


def build_nc():
    """Build the SPMD Bass program (same program on every core)."""
    nc = bacc.Bacc("TRN2", target_bir_lowering=False, debug=False, num_devices=N_CORES)

    xT = nc.dram_tensor("xT", [HIDDEN, T_CORE], BF16, kind="ExternalInput").ap()
    wT = nc.dram_tensor("wT", [HIDDEN, OUT], BF16, kind="ExternalInput").ap()
    aT = nc.dram_tensor("aT", [HIDDEN, 3 * GR], BF16, kind="ExternalInput").ap()
    bqkv = nc.dram_tensor("bqkv", [GR, OUT], BF16, kind="ExternalInput").ap()
    mask = nc.dram_tensor("mask", [GR, T_CORE], BF16, kind="ExternalInput").ap()
    yb = nc.dram_tensor("yb", [T_CORE, OUT], BF16, kind="ExternalOutput").ap()

    with tile.TileContext(nc) as tc:
        with (
            tc.tile_pool(name="xsb", bufs=1) as xpool,
            tc.tile_pool(name="wsb", bufs=1) as wpool,
            tc.tile_pool(name="asb", bufs=1) as apool,
            tc.tile_pool(name="bsb", bufs=1) as bpool,
            tc.tile_pool(name="msb", bufs=1) as mpool,
            tc.tile_pool(name="ht", bufs=1) as htpool,
            tc.tile_pool(name="o", bufs=12) as opool,
            tc.tile_pool(name="ps", bufs=1, space="PSUM") as pspool,
        ):
            # --- resident inputs (DMA'd in chunks; tile deps order the MMs) ---
            # Issue order matters: phase 1 streams x/A chunks immediately;
            # mask+B are needed at the first delta (~+35us); w chunk j is
            # needed only when phase 2 reaches o-chunk j (j-outer loop).
            xsb = xpool.tile([P, KC * T_CORE], BF16)    # free idx = k*1024 + t
            asb = apool.tile([P, KC * 3 * GR], BF16)    # free idx = k*384 + g*128 + lr
            for k in range(KC):
                nc.sync.dma_start(
                    xsb[:, k * T_CORE:(k + 1) * T_CORE], xT[k * P:(k + 1) * P, :])
                nc.sync.dma_start(
                    asb[:, k * 3 * GR:(k + 1) * 3 * GR], aT[k * P:(k + 1) * P, :])
            msb = mpool.tile([P, T_CORE], BF16)         # routing mask [lr, t]
            nc.sync.dma_start(msb[:], mask[:, :])
            bsb = bpool.tile([P, OUT], BF16)            # [lr, o] = bq|bk|bv
            nc.sync.dma_start(bsb[:], bqkv[:, :])
            # w stored per o-chunk: free idx = j*(16*512) + k*512 + o
            wsb = wpool.tile([P, KC * OUT], BF16)
            for j in range(OJ):
                for k in range(KC):
                    nc.sync.dma_start(
                        wsb[:, (j * KC + k) * 512:(j * KC + k + 1) * 512],
                        wT[k * P:(k + 1) * P, j * 512:(j + 1) * 512])

            # --- phase 1: hT[g][lr, t], A-chunk stationary, x moving ---
            hps = [pspool.tile([P, 512], F32, name=f"hps{n}", tag=f"ps{n}") for n in range(6)]
            for k in range(KC):
                for g in range(3):
                    for th in range(2):
                        nc.tensor.matmul(
                            hps[g * 2 + th][:],
                            lhsT=asb[:, k * 3 * GR + g * GR:
                                     k * 3 * GR + (g + 1) * GR],
                            rhs=xsb[:, k * T_CORE + th * 512:
                                    k * T_CORE + (th + 1) * 512],
                            start=(k == 0), stop=(k == KC - 1))
            # mask (one-hot) and downcast to bf16 for the delta matmuls
            hT = htpool.tile([P, 3 * T_CORE], BF16)     # free idx = g*1024 + t
            for g in range(3):
                for th in range(2):
                    nc.vector.tensor_tensor(
                        hT[:, g * T_CORE + th * 512:g * T_CORE + (th + 1) * 512],
                        hps[g * 2 + th][:], msb[:, th * 512:(th + 1) * 512],
                        op=mybir.AluOpType.mult)

            # --- phase 2: y tile [t128, o512]; j-outer so w chunk j is only
            # needed once phase 2 reaches it (relaxes the DMA deadline) ---
            # delta source group per o-chunk j: q,q,q,q,k,v
            jg = [0, 0, 0, 0, 1, 2]
            for j in range(OJ):
                for i in range(NT):
                    n = j * NT + i
                    ops = pspool.tile([P, 512], F32, name=f"ops{n}",
                                      tag=f"ps{n % 8}")
                    for k in range(KC):
                        nc.tensor.matmul(
                            ops[:],
                            lhsT=xsb[:, k * T_CORE + i * P:
                                     k * T_CORE + (i + 1) * P],
                            rhs=wsb[:, (j * KC + k) * 512:
                                    (j * KC + k + 1) * 512],
                            start=(k == 0), stop=False)
                    nc.tensor.matmul(
                        ops[:],
                        lhsT=hT[:, jg[j] * T_CORE + i * P:
                                jg[j] * T_CORE + (i + 1) * P],
                        rhs=bsb[:, j * 512:(j + 1) * 512],
                        start=False, stop=True)
                    osb = opool.tile([P, 512], BF16, name=f"osb{i}_{j}", tag="o")
                    if n % 2 == 0:
                        nc.scalar.copy(osb[:], ops[:])
                    else:
                        nc.vector.tensor_copy(osb[:], ops[:])
                    nc.sync.dma_start(
                        yb[i * P:(i + 1) * P, j * 512:(j + 1) * 512], osb[:])
    nc.compile()
    return nc


def prep_in_maps(x, weight, lora_A, lora_B_q, lora_B_k, lora_B_v,
                 lora_scaling, token_to_slot):
    x = np.asarray(x, dtype=np.float32)
    weight = np.asarray(weight, dtype=np.float32)
    lora_A = np.asarray(lora_A, dtype=np.float32)
    lora_B_q = np.asarray(lora_B_q, dtype=np.float32)
    lora_B_k = np.asarray(lora_B_k, dtype=np.float32)
    lora_B_v = np.asarray(lora_B_v, dtype=np.float32)
    lora_scaling = np.asarray(lora_scaling, dtype=np.float32)
    slot = np.asarray(token_to_slot).astype(np.int64)

    xT = np.ascontiguousarray(x.astype(NPBF16).T)       # (2048, 8192) bf16
    wT = np.ascontiguousarray(weight.astype(NPBF16).T)  # (2048, 3072) bf16
    # aT col = g*128 + l*16 + r
    aT = np.ascontiguousarray(
        lora_A.transpose(1, 0, 2, 3).reshape(3 * GR, HIDDEN).T.astype(NPBF16))
    # b row = l*16 + r, scaling folded in; columns = q | k | v
    bq = (lora_scaling[:, None, None] * lora_B_q).transpose(0, 2, 1).reshape(GR, Q_SIZE)
    bk = (lora_scaling[:, None, None] * lora_B_k).transpose(0, 2, 1).reshape(GR, KV_SIZE)
    bv = (lora_scaling[:, None, None] * lora_B_v).transpose(0, 2, 1).reshape(GR, KV_SIZE)
    bqkv = np.ascontiguousarray(
        np.concatenate([bq, bk, bv], axis=1).astype(NPBF16))  # (128, 3072)
    # routing mask [l*16+r, t]: 1 where slot[t] == l (scale already in B)
    onehot = (slot[None, :] == np.arange(MAX_LORAS)[:, None])          # (8, T)
    mask = np.repeat(onehot, RANK, axis=0).astype(NPBF16)              # (128, T)

    in_maps = []
    for c in range(N_CORES):
        sl = slice(c * T_CORE, (c + 1) * T_CORE)
        in_maps.append({
            "xT": np.ascontiguousarray(xT[:, sl]),
            "wT": wT,
            "aT": aT,
            "bqkv": bqkv,
            "mask": np.ascontiguousarray(mask[:, sl]),
        })
    return in_maps


def kernel(**inputs):
    from concourse.bass_utils import run_bass_kernel_spmd
    if "nc" not in _NC_CACHE:
        _NC_CACHE["nc"] = build_nc()
    nc = _NC_CACHE["nc"]
    in_maps = prep_in_maps(**inputs)
    res = run_bass_kernel_spmd(nc, in_maps, core_ids=list(range(N_CORES)))
    return np.concatenate(
        [r["yb"].astype(np.float32) for r in res.results], axis=0)
